# revision 23
# baseline (speedup 1.0000x reference)
"""Trainium2 Bass kernel for a fused LSTM cell — mixed bf16/fp8 edition.

Problem: B=8192, I=H=1024.
  gates = [x, h_prev] @ [W_f|W_i|W_o|W_C] + b      # [B, 4H]
  C_t = sigmoid(f)*C_prev + sigmoid(i)*tanh(c)
  h_t = sigmoid(o)*tanh(C_t)

Sharding: data-parallel over batch across 8 NeuronCores (1024 rows each),
weights replicated. No collectives needed.

Per-core device program:
  - Per-gate matmul precision (GATES config). bf16 gates: 16 matmuls of
    K=128 per (q,m) block at full PE rate. fp8 gates: e4m3 with
    MatmulPerfMode.DoubleRow — K=256 per instruction at 2x rate (half
    the PE cycles of bf16) — plus optional residual-correction passes at
    a shared scale. Default: i in single-pass fp8 (its error is damped
    by the |tanh(C~)| <= 1 factor in C_t = f*C_prev + i*C~), f/o/C in
    bf16. W pre-scaled by 64 for fp8 so values clear the e4m3 subnormal
    floor; the ScalarE activation's scale operand undoes it for fp8
    gates (out = func(psum*scale + bias)).
  - Hidden dim on PSUM partitions so the per-gate bias rides the ScalarE
    activation per-partition bias operand.
  - Loop q (8 H-chunks of 128) x m (2 batch-chunks of 512): 4 gate chains
    into 4 PSUM banks, ScalarE sigmoid/tanh eviction, VectorE elementwise
    for C_t / h_t, DMA out in [H, B] layout (untransposed on host).

All host-side layout shuffles (transpose/quantize/reorder) are numpy
copies outside the measured device execution.
"""

import numpy as np
import ml_dtypes

import concourse.bass as bass
import concourse.mybir as mybir
import concourse.tile as tile
from concourse import bacc
from concourse.bass_utils import run_bass_kernel_spmd

N_CORES = 8
B, I, H = 8192, 1024, 1024
K = I + H                      # 2048 contraction dim
BL = B // N_CORES              # 1024 batch rows per core
KC = K // 128                  # 16 K-chunks (bf16 path)
KC2 = K // 256                 # 8 double-row K-chunks (fp8 path)
QC = H // 128                  # 8 hidden chunks of 128
MC = 2                         # batch chunks of 512 per core
MT = BL // MC                  # 512
NCHUNKS = 4 * QC               # 32 (q-major, gate-minor) N-chunks of 128

_DT8 = mybir.dt.float8e4
_NP8 = ml_dtypes.float8_e4m3
_DTB = mybir.dt.bfloat16
_NPB = ml_dtypes.bfloat16
SW = 64.0                      # fp8 weight pre-scale
_DR = mybir.MatmulPerfMode.DoubleRow

# chain order within a group: f, i, C~ (tanh), o — o last so the final
# epilogue's critical path after the last matmul is just sigmoid(o)*tanh(C_t)
GATE_ORDER = (0, 1, 3, 2)      # index into (f, i, o, C) weight slots

# per-gate precision: 'fp32r', 'bf16', or ('fp8', passes) with passes a
# subset of ('hh','hl','lh'): hh=xh@wh, hl=xh@wl (W resid), lh=xl@wh (x resid)
GATES = {
    "f": "fp32r",
    "i": ("fp8", ("hh",)),
    "o": "fp32r",
    "C": "fp32r",
}

_SIG = mybir.ActivationFunctionType.Sigmoid
_TANH = mybir.ActivationFunctionType.Tanh


def set_gates(cfg: dict):
    """Test-only knob."""
    global GATES
    GATES = dict(cfg)
    _NC_CACHE.clear()


def _gname(g):
    return ("f", "i", "o", "C")[GATE_ORDER[g]]


def build_program(repeats: int = 1):
    """Build the per-core Bass program. `repeats` unrolls the whole body
    (same data) for slope-based HW timing in test harnesses."""
    nc = bacc.Bacc("TRN2", target_bir_lowering=False, debug=False)

    used16 = {GATES[_gname(g)] for g in range(4)
              if GATES[_gname(g)] in ("bf16", "fp32r")}
    fp8_lvls = set()
    for g in range(4):
        cfg = GATES[_gname(g)]
        if not isinstance(cfg, str):
            for p in cfg[1]:
                fp8_lvls.add(p[0])  # x level used
    need_x8h = "h" in fp8_lvls
    need_x8l = "l" in fp8_lvls
    _DT16 = {"bf16": _DTB, "fp32r": mybir.dt.float32r}

    # Host-prepped layouts (see prep_inputs):
    #   combb:       [128, KC, MC, MT] bf16 combined^T
    #   comb8h/8l:   [128, KC2, MC, 2, MT] fp8 combined^T hi/lo
    #   wb:          [NCHUNKS, 128, KC, 128] bf16 W tiles (c = q*4+gate)
    #   w8h/w8l:     [NCHUNKS, 128, KC2, 2, 128] fp8 W tiles
    #   bt:          [128, NCHUNKS]  bias chunks
    #   cp:          [128, QC, BL]   C_prev^T
    any_fp8 = any(not isinstance(GATES[_gname(g)], str) for g in range(4))
    need_w8l = any(
        not isinstance(GATES[_gname(g)], str)
        and any(p[1] == "l" for p in GATES[_gname(g)][1])
        for g in range(4)
    )
    comb16_d = {
        dtn: nc.dram_tensor(f"comb_{dtn}", [128, KC, MC, MT], _DT16[dtn],
                            kind="ExternalInput")
        for dtn in sorted(used16)
    }
    comb8h_d = (nc.dram_tensor("comb8h", [128, KC2, MC, 2, MT], _DT8, kind="ExternalInput")
                if need_x8h else None)
    comb8l_d = (nc.dram_tensor("comb8l", [128, KC2, MC, 2, MT], _DT8, kind="ExternalInput")
                if need_x8l else None)
    w16_d = {
        dtn: nc.dram_tensor(f"w_{dtn}", [NCHUNKS, 128, KC, 128], _DT16[dtn],
                            kind="ExternalInput")
        for dtn in sorted(used16)
    }
    w8h_d = (nc.dram_tensor("w8h", [NCHUNKS, 128, KC2, 2, 128], _DT8, kind="ExternalInput")
             if any_fp8 else None)
    w8l_d = (nc.dram_tensor("w8l", [NCHUNKS, 128, KC2, 2, 128], _DT8, kind="ExternalInput")
             if need_w8l else None)
    bt_d = nc.dram_tensor("bt", [128, NCHUNKS], mybir.dt.float32, kind="ExternalInput")
    cp_d = nc.dram_tensor("cp", [128, QC, BL], mybir.dt.float32, kind="ExternalInput")
    ht_d = nc.dram_tensor("ht", [QC, 128, BL], mybir.dt.float32, kind="ExternalOutput")
    ct_d = nc.dram_tensor("ct", [QC, 128, BL], mybir.dt.float32, kind="ExternalOutput")

    with tile.TileContext(nc) as tc:
        with (
            tc.tile_pool(name="res", bufs=1) as res,
            tc.tile_pool(name="wp", bufs=12) as wp,
            tc.tile_pool(name="cpp", bufs=2) as cpp,
            tc.tile_pool(name="gp", bufs=2) as gp,
            tc.tile_pool(name="ep", bufs=2) as ep,
            tc.tile_pool(name="psum", bufs=2, space="PSUM") as pp,
        ):
            combs_16 = {}  # (k, m, dtn) -> [128, MT] tile
            combs_8 = {}   # (kk, m, lvl) -> [128, 2, MT] fp8 tile

            def _load_comb_16(k, m, dtn):
                t = res.tile([128, MT], _DT16[dtn], tag=f"cb{dtn}{k}_{m}",
                             name=f"cb{dtn}{k}_{m}")
                nc.sync.dma_start(out=t[:], in_=comb16_d[dtn].ap()[:, k, m, :])
                combs_16[(k, m, dtn)] = t

            def _load_comb_8(kk, m, lvl):
                src = comb8h_d if lvl == "h" else comb8l_d
                t = res.tile([128, 2, MT], _DT8, tag=f"c8{lvl}{kk}_{m}",
                             name=f"c8{lvl}{kk}_{m}")
                nc.sync.dma_start(out=t[:], in_=src.ap()[:, kk, m, :, :])
                combs_8[(kk, m, lvl)] = t

            def _load_w(q, g, ksplit=2, tagsuf=""):
                """W tiles for (q-chunk, gate): list of parts along K."""
                cfg = GATES[_gname(g)]
                c = q * 4 + GATE_ORDER[g]
                out = {}
                if isinstance(cfg, str):
                    step = KC // ksplit
                    parts = []
                    for s in range(ksplit):
                        wt = wp.tile([128, step, 128], _DT16[cfg],
                                     tag=f"w{cfg}{tagsuf}",
                                     name=f"w{cfg}{q}_{g}_{s}")
                        nc.sync.dma_start(
                            out=wt[:],
                            in_=w16_d[cfg].ap()[c, :, s * step:(s + 1) * step, :],
                        )
                        parts.append(wt)
                    out["b"] = (step, parts)
                else:
                    lvls = {p[1] for p in cfg[1]}
                    for lvl in sorted(lvls):
                        src = w8h_d if lvl == "h" else w8l_d
                        step = KC2 // min(ksplit, KC2)
                        parts = []
                        for s in range(KC2 // step):
                            wt = wp.tile([128, step, 2, 128], _DT8,
                                         tag=f"w8{lvl}{tagsuf}",
                                         name=f"w8{lvl}{q}_{g}_{s}")
                            nc.sync.dma_start(
                                out=wt[:],
                                in_=src.ap()[c, :, s * step:(s + 1) * step, :, :],
                            )
                            parts.append(wt)
                        out[lvl] = (step, parts)
                return out

            def _emit_chain(g, m, ps_t, wts_g):
                """All matmuls for gate g into PSUM tile ps_t."""
                cfg = GATES[_gname(g)]
                if isinstance(cfg, str):
                    step, parts = wts_g["b"]
                    for k in range(KC):
                        nc.tensor.matmul(
                            ps_t[:],
                            lhsT=parts[k // step][:, k % step, :],
                            rhs=combs_16[(k, m, cfg)][:],
                            start=(k == 0),
                            stop=(k == KC - 1),
                        )
                else:
                    passes = cfg[1]
                    nmm = len(passes) * KC2
                    j = 0
                    for pss in passes:
                        xl_, wl_ = pss[0], pss[1]
                        step, parts = wts_g[wl_]
                        for kk in range(KC2):
                            nc.tensor.matmul(
                                ps_t[:],
                                lhsT=parts[kk // step][:, kk % step, :, :],
                                rhs=combs_8[(kk, m, xl_)][:],
                                start=(j == 0),
                                stop=(j == nmm - 1),
                                perf_mode=_DR,
                            )
                            j += 1

            # --- one-time loads, ordered by first consumption ---
            wts = {}
            wts[0] = _load_w(0, 0, ksplit=2)
            for dtn in sorted(used16):
                for k in range(KC):
                    _load_comb_16(k, 0, dtn)
            if need_x8h:
                for kk in range(KC2):
                    _load_comb_8(kk, 0, "h")
            if need_x8l:
                for kk in range(KC2):
                    _load_comb_8(kk, 0, "l")
            wts[1] = _load_w(0, 1, ksplit=2)
            bt_sb = res.tile([128, NCHUNKS], mybir.dt.float32)
            nc.sync.dma_start(out=bt_sb[:], in_=bt_d.ap())
            wts[2] = _load_w(0, 2, ksplit=2)
            wts[3] = _load_w(0, 3, ksplit=2)
            for dtn in sorted(used16):
                for k in range(KC):
                    _load_comb_16(k, 1, dtn)
            if need_x8h:
                for kk in range(KC2):
                    _load_comb_8(kk, 1, "h")
            if need_x8l:
                for kk in range(KC2):
                    _load_comb_8(kk, 1, "l")

            for _ in range(repeats):
                for q in range(QC):
                    if q > 0:
                        wts = {g: _load_w(q, g, ksplit=2) for g in range(4)}
                    for m in range(MC):
                        ms = slice(m * MT, (m + 1) * MT)
                        ps = [
                            pp.tile([128, MT], mybir.dt.float32, name=f"ps{g}", tag=f"ps{g}")
                            for g in range(4)
                        ]
                        # g-outer: each gate's chain completes after only its
                        # own W tiles; its activation overlaps later chains
                        for g in range(4):
                            _emit_chain(g, m, ps[g], wts[g])
                        # epilogue: chains finish in order f,i,C,o; o's
                        # sigmoid + final mul are the only ops after the last
                        # matmul of the group. cp load emitted after the MMs so
                        # W tiles keep DMA queue priority.
                        cp_t = cpp.tile([128, MT], mybir.dt.float32, tag="cp")
                        nc.sync.dma_start(out=cp_t[:], in_=cp_d.ap()[:, q, ms])
                        c0 = q * 4
                        scl = [1.0 if isinstance(GATES[_gname(g)], str) else 1.0 / SW
                               for g in range(4)]
                        f_sb = gp.tile([128, MT], mybir.dt.float32, tag="f", name="f_sb")
                        i_sb = gp.tile([128, MT], mybir.dt.float32, tag="i", name="i_sb")
                        o_sb = gp.tile([128, MT], mybir.dt.float32, tag="o", name="o_sb")
                        cl_sb = gp.tile([128, MT], mybir.dt.float32, tag="cl", name="cl_sb")
                        nc.scalar.activation(f_sb[:], ps[0][:], _SIG,
                                             bias=bt_sb[:, c0:c0 + 1], scale=scl[0])
                        nc.scalar.activation(i_sb[:], ps[1][:], _SIG,
                                             bias=bt_sb[:, c0 + 1:c0 + 2], scale=scl[1])
                        nc.scalar.activation(cl_sb[:], ps[2][:], _TANH,
                                             bias=bt_sb[:, c0 + 3:c0 + 4], scale=scl[2])
                        # C_t = f*C_prev + i*ctilda ; h_t = o*tanh(C_t)
                        t1 = ep.tile([128, MT], mybir.dt.float32, tag="t1", name="t1")
                        t2 = ep.tile([128, MT], mybir.dt.float32, tag="t2", name="t2")
                        c_out = ep.tile([128, MT], mybir.dt.float32, tag="c_out", name="c_out")
                        th = ep.tile([128, MT], mybir.dt.float32, tag="th", name="th")
                        h_out = ep.tile([128, MT], mybir.dt.float32, tag="h_out", name="h_out")
                        nc.vector.tensor_tensor(t1[:], f_sb[:], cp_t[:], mybir.AluOpType.mult)
                        nc.vector.tensor_tensor(t2[:], i_sb[:], cl_sb[:], mybir.AluOpType.mult)
                        nc.vector.tensor_tensor(c_out[:], t1[:], t2[:], mybir.AluOpType.add)
                        nc.scalar.activation(th[:], c_out[:], _TANH)
                        nc.sync.dma_start(out=ct_d.ap()[q, :, ms], in_=c_out[:])
                        last = q == QC - 1 and m == MC - 1
                        if last:
                            # split the final o->h chain so ACT/DVE/DMA overlap
                            # after the very last matmul
                            hw_ = MT // 2
                            for s in range(2):
                                sl = slice(s * hw_, (s + 1) * hw_)
                                osl = slice(m * MT + s * hw_, m * MT + (s + 1) * hw_)
                                nc.scalar.activation(
                                    o_sb[:, sl], ps[3][:, sl], _SIG,
                                    bias=bt_sb[:, c0 + 2:c0 + 3], scale=scl[3],
                                )
                                nc.vector.tensor_tensor(
                                    h_out[:, sl], o_sb[:, sl], th[:, sl],
                                    mybir.AluOpType.mult,
                                )
                                nc.sync.dma_start(out=ht_d.ap()[q, :, osl], in_=h_out[:, sl])
                        else:
                            nc.scalar.activation(o_sb[:], ps[3][:], _SIG,
                                                 bias=bt_sb[:, c0 + 2:c0 + 3], scale=scl[3])
                            nc.vector.tensor_tensor(h_out[:], o_sb[:], th[:],
                                                    mybir.AluOpType.mult)
                            nc.sync.dma_start(out=ht_d.ap()[q, :, ms], in_=h_out[:])
    nc.compile()
    return nc


def prep_inputs(x, h_prev, C_prev, W_f, b_f, W_i, b_i, W_C, b_C, W_o, b_o):
    """Shard + quantize + lay out host arrays. Returns in_maps."""
    f32 = np.float32
    x = np.ascontiguousarray(x, f32)
    h_prev = np.ascontiguousarray(h_prev, f32)
    C_prev = np.ascontiguousarray(C_prev, f32)

    any_fp8 = any(not isinstance(GATES[g], str) for g in "fioC")
    used16 = {GATES[g] for g in "fioC" if isinstance(GATES[g], str)}
    np16 = {"bf16": _NPB, "fp32r": f32}
    # 16-chunk W tiles: [QC, 4, 128(p), KC, 128(n)]
    w5 = np.empty((QC, 4, 128, KC, 128), f32)
    for g, Wg in enumerate((W_f, W_i, W_o, W_C)):
        Wg = np.ascontiguousarray(Wg, f32)
        w5[:, g] = Wg.reshape(KC, 128, QC, 128).transpose(2, 1, 0, 3)
    w16 = {dtn: np.ascontiguousarray(
               w5.reshape(NCHUNKS, 128, KC, 128).astype(np16[dtn]))
           for dtn in used16}
    w8h = w8l = np.zeros(1, _NP8)
    if any_fp8:
        # fp8 W tiles: same data * SW in DoubleRow layout [.., KC2, 2, 128]
        w6 = (w5.reshape(NCHUNKS, 128, KC2, 2, 128) * SW)
        w8h = w6.astype(_NP8)
        w8l = (w6 - w8h.astype(f32)).astype(_NP8)

    bt = np.empty((QC, 4, 128), f32)
    for g, bg in enumerate((b_f, b_i, b_o, b_C)):
        bt[:, g] = np.asarray(bg, f32).reshape(QC, 128)
    bt = np.ascontiguousarray(bt.reshape(NCHUNKS, 128).T)  # [128, NCHUNKS]

    in_maps = []
    for c in range(N_CORES):
        rs = slice(c * BL, (c + 1) * BL)
        comb = np.empty((K, BL), f32)
        comb[:I] = x[rs].T
        comb[I:] = h_prev[rs].T
        # 16-chunk: [p, k, m, t] = comb[k*128+p, m*MT+t]
        cb4 = comb.reshape(KC, 128, MC, MT).transpose(1, 0, 2, 3)
        c16 = {dtn: np.ascontiguousarray(cb4.astype(np16[dtn]))
               for dtn in used16}
        c8h = c8l = np.zeros(1, _NP8)
        if any_fp8:
            # fp8: [p, kk, m, j, t] = comb[kk*256 + j*128 + p, m*MT+t]
            cr = comb.reshape(KC2, 2, 128, MC, MT).transpose(2, 0, 3, 1, 4)
            c8h = cr.astype(_NP8)
            c8l = np.ascontiguousarray((cr - c8h.astype(f32)).astype(_NP8))
            c8h = np.ascontiguousarray(c8h)
        cp = np.ascontiguousarray(
            C_prev[rs].T.reshape(QC, 128, BL).transpose(1, 0, 2)
        )
        im = {"comb8h": c8h, "comb8l": c8l, "w8h": w8h, "w8l": w8l,
              "bt": bt, "cp": cp}
        for dtn in used16:
            im[f"comb_{dtn}"] = c16[dtn]
            im[f"w_{dtn}"] = w16[dtn]
        in_maps.append(im)
    return in_maps


def assemble_outputs(results):
    """Gather per-core [QC, 128, BL] outputs into full [B, H] h_t, C_t."""
    h_t = np.empty((B, H), np.float32)
    C_t = np.empty((B, H), np.float32)
    for c, r in enumerate(results):
        rs = slice(c * BL, (c + 1) * BL)
        h_t[rs] = r["ht"].reshape(H, BL).T
        C_t[rs] = r["ct"].reshape(H, BL).T
    return h_t, C_t


_NC_CACHE = {}


def kernel(**inputs):
    if "nc" not in _NC_CACHE:
        _NC_CACHE["nc"] = build_program(repeats=1)
    nc = _NC_CACHE["nc"]
    in_maps = prep_inputs(**inputs)
    res = run_bass_kernel_spmd(nc, in_maps, core_ids=list(range(N_CORES)))
    return assemble_outputs(res.results)
